# revision 5
# baseline (speedup 1.0000x reference)
"""Trainium2 Bass kernel for nn_Conv2d_NN_spatial (retrieval_knn).

Full-input contract: kernel(**inputs) takes the unsharded inputs and returns
the full output. Internally: data-parallel over batch across 8 NeuronCores
(4 batches per core).

Per-core algorithm:
  1. negd2 = 2*dot(x2, xs) - m2 via THREE accumulating bf16 matmuls per
     128-token chunk (hi/lo split: Xh·Sh + Xh·Sl + Xl·Sh; the dropped
     Xl·Sl term is ~2^-16 relative -- measured end-to-end rel err 7.2e-3).
     This runs 1 cyc/row vs fp32's 4 cyc/row on the PE.
  2. top-3 neighbors per token via DVE max (top-8) + max_index directly on
     the PSUM distance tile (tie-break == jax top_k).
  3. Projected tables P_k = W_k @ xs + bias/3 (o-permuted padded weights),
     spread into j-slices of R and folded 4->1 across partitions by a
     delta-matrix matmul: TABQ holds, per partition (b,oquad,tokhalf), rows
     [(k,m), 4 o-values] bf16 -- no table DMA.
  4. GPSIMD ap_gather with d=4 fetches 4 o-channels per index, 6144 indices
     per gpsimd core, SPLIT into 8 quarter-gathers pipelined against the
     distance/top-k loop (split-major processing order).  Idx streams
     round-trip DRAM in [lane][word] layout so fold DMAs stay coarse.
  5. Sum the 3 gathered projections per split (2 DVE strided bf16 adds,
     2x mode) -> out bf16; host converts to fp32.
Pixel unshuffle/shuffle are pure layout transforms done host-side.
"""
import numpy as np
import ml_dtypes

import concourse.bacc as bacc
import concourse.bass as bass
import concourse.mybir as mybir
import concourse.tile as tile
from concourse.bass_utils import run_bass_kernel_spmd

F32 = mybir.dt.float32
BF16 = mybir.dt.bfloat16
U16 = mybir.dt.uint16
I16 = mybir.dt.int16

N_CORES = 8
B_PER_CORE = 4
C1 = 64          # unshuffled channels
N = 4096         # tokens per batch (64*64)
NHALF = N // 2   # tokens per gather group (th = token half)
M = 256          # samples
K = 3
NIDX = NHALF * K  # gather indices per gpsimd core (6144)
NW = NIDX // 16   # idx words per lane (384)
NSPLIT = 8        # pipelined gather splits
CPS = 16 // NSPLIT  # chunks-per-half per split (2)
WPS = NW // NSPLIT  # idx words per split (48)
IPS = NIDX // NSPLIT  # idx per split per core (768)

SIDX = [0, 4, 8, 13, 17, 21, 25, 29, 34, 38, 42, 46, 50, 55, 59, 63]
SAMPLE_FLAT = (np.array(SIDX)[:, None] * 64 + np.array(SIDX)[None, :]).reshape(-1)

_CACHE = {}


def build_program():
    """Build the per-core Bass program (SPMD: same program on all 8 cores)."""
    nc = bacc.Bacc("TRN2", target_bir_lowering=False, debug=False,
                   enable_asserts=False)

    # hi/lo bf16 splits of [x2; 1] (65 rows) and [2*xs; -m2] (65 rows)
    xhe = nc.dram_tensor("xhe", [B_PER_CORE, 65, N], BF16, kind="ExternalInput")
    xle = nc.dram_tensor("xle", [B_PER_CORE, 65, N], BF16, kind="ExternalInput")
    she = nc.dram_tensor("she", [B_PER_CORE, 65, M], BF16, kind="ExternalInput")
    sle = nc.dram_tensor("sle", [B_PER_CORE, 65, M], BF16, kind="ExternalInput")
    # fp32 sample matrix for table projection only: rows 0-63 = 2*xs, 65 = 1
    xse = nc.dram_tensor("xse", [B_PER_CORE, 66, M], F32, kind="ExternalInput")
    wke = nc.dram_tensor("wke", [K, 66, 128], F32, kind="ExternalInput")
    # delta fold matrices: [b][o'' 128][psum col 128] bf16
    dle = nc.dram_tensor("dle", [B_PER_CORE, 128, 128], BF16,
                         kind="ExternalInput")
    # rank offsets 0,256,512 tiled: [128, 2*CPS*K] u16
    offe = nc.dram_tensor("offe", [128, 2 * CPS * K], U16, kind="ExternalInput")
    outd = nc.dram_tensor("outd", [128, 2 * N], BF16, kind="ExternalOutput")
    # idx stream scratch in [lane][word] layout:
    # stream step i = ((cp*8+pg)*3 + k)*16 + pl -> lane pl, word cp*24+pg*3+k
    idxscr = nc.dram_tensor("idxscr", [NSPLIT, B_PER_CORE, 128, 2 * CPS * K],
                            U16, kind="Internal")

    AA = mybir.AluOpType

    with tile.TileContext(nc) as tc:
        with (
            tc.tile_pool(name="xp", bufs=4) as xp,
            tc.tile_pool(name="sp", bufs=4) as sp,
            tc.tile_pool(name="cst", bufs=1) as cst,
            tc.tile_pool(name="tbp", bufs=2) as tbp,
            tc.tile_pool(name="m8p", bufs=4) as m8p,
            tc.tile_pool(name="ixp", bufs=3) as ixp,
            tc.tile_pool(name="gp", bufs=1) as gp,
            tc.tile_pool(name="op", bufs=2) as op_,
            tc.tile_pool(name="ps", bufs=4, space=bass.MemorySpace.PSUM) as psp,
            tc.tile_pool(name="ps2", bufs=2, space=bass.MemorySpace.PSUM) as psp2,
            tc.tile_pool(name="ps3", bufs=2, space=bass.MemorySpace.PSUM) as psp3,
        ):
            OFFS = cst.tile([128, 2 * CPS * K], U16, tag="OFFS")
            nc.sync.dma_start(OFFS[:], offe[:])
            wk = []
            for k in range(K):
                t = cst.tile([66, 128], F32, tag=f"wk{k}")
                nc.sync.dma_start(t[:], wke[k])
                wk.append(t)
            dl = []
            for b in range(B_PER_CORE):
                t = cst.tile([128, 128], BF16, tag=f"dl{b}")
                nc.sync.dma_start(t[:], dle[b])
                dl.append(t)

            # gather table: partition (b*32 + th*16 + oq), free (k, m, j)
            TABQ = gp.tile([128, K * M * 4], BF16, tag="TABQ")

            # ---- load S (tables) + hi/lo split matrices; preload all X ----
            S_tiles, Sh_tiles, Sl_tiles = [], [], []
            for b in range(B_PER_CORE):
                S = sp.tile([66, M], F32, tag="S")
                nc.sync.dma_start(S[:], xse[b])
                S_tiles.append(S)
                Sh = sp.tile([65, M], BF16, tag="Sh")
                nc.sync.dma_start(Sh[:], she[b])
                Sh_tiles.append(Sh)
                Sl = sp.tile([65, M], BF16, tag="Sl")
                nc.sync.dma_start(Sl[:], sle[b])
                Sl_tiles.append(Sl)
            Xh_tiles, Xl_tiles = [], []
            for b in range(B_PER_CORE):
                Xh = xp.tile([65, N], BF16, tag="Xh")
                nc.sync.dma_start(Xh[:], xhe[b])
                Xh_tiles.append(Xh)
                Xl = xp.tile([65, N], BF16, tag="Xl")
                nc.scalar.dma_start(Xl[:], xle[b])
                Xl_tiles.append(Xl)

            # R[o'',(k,m,j)] zeroed ONCE; each batch rewrites the same
            # nonzero cells (j == o''//32 slices), zeros elsewhere persist.
            R = tbp.tile([128, K * M * 4], BF16, tag="R")
            nc.vector.memset(R[:], 0.0)
            rv = R[:].rearrange("p (r j) -> p r j", j=4)

            def emit_tables_b(b):
                # All table elementwise work on the (idle) scalar engine so
                # the DVE stays free for the top-k stream.
                S = S_tiles[b]
                # P''-stack [128 o''=(j*32+oq), (k, m)] bf16
                PS = tbp.tile([128, K * M], BF16, tag="PS")
                for k in range(K):
                    tp = psp2.tile([128, M], F32, tag="tabps")
                    nc.tensor.matmul(tp[:], wk[k][:], S[:])
                    nc.scalar.copy(PS[:, k * M:(k + 1) * M], tp[:])
                for j in range(4):
                    nc.scalar.copy(
                        rv[j * 32:j * 32 + 16, :, j:j + 1],
                        PS[j * 32:j * 32 + 16, :].rearrange(
                            "p (r u) -> p r u", u=1))
                # fold 4->1 partitions: TABQ[b*32+th*16+oq] = P[oq*4+j]
                for ch in range(6):
                    tq = psp3.tile([128, 512], F32, tag="tqps")
                    nc.tensor.matmul(tq[:], dl[b][:],
                                     R[:, ch * 512:(ch + 1) * 512])
                    nc.scalar.copy(
                        TABQ[b * 32:(b + 1) * 32,
                             ch * 512:(ch + 1) * 512],
                        tq[b * 32:(b + 1) * 32, :])

            # ---- distance + top-3, split-major for gather pipelining ----
            G = gp.tile([128, NIDX * 4], BF16, tag="G")
            WIDX = cst.tile([128, NW], I16, tag="WIDX")
            for s in range(NSPLIT):
                for b in range(B_PER_CORE):
                    Xh, Xl = Xh_tiles[b], Xl_tiles[b]
                    Sh, Sl = Sh_tiles[b], Sl_tiles[b]
                    IDXS = ixp.tile([128, 2 * CPS * 8], U16, tag="IDXS")
                    for ci in range(2 * CPS):
                        th, cp = ci // CPS, ci % CPS
                        gci = th * 16 + s * CPS + cp  # global chunk id
                        xsl = slice(gci * 128, (gci + 1) * 128)
                        nd = psp.tile([128, M], F32, tag="nd")
                        nc.tensor.matmul(nd[:], Xh[:, xsl], Sh[:],
                                         start=True, stop=False)
                        nc.tensor.matmul(nd[:], Xh[:, xsl], Sl[:],
                                         start=False, stop=False)
                        nc.tensor.matmul(nd[:], Xl[:, xsl], Sh[:],
                                         start=False, stop=True)
                        M8 = m8p.tile([128, 8], F32, tag="M8")
                        nc.vector.max(M8[:], nd[:])
                        nc.vector.max_index(IDXS[:, ci * 8:(ci + 1) * 8],
                                            M8[:], nd[:])

                    # slice k<3, add 256*k table-row offset (one DVE op)
                    IDXC = ixp.tile([128, 2 * CPS * K], U16, tag="IDXC")
                    src = IDXS[:].rearrange("p (c e) -> p c e", e=8)
                    dst = IDXC[:].rearrange("p (c e) -> p c e", e=3)
                    ofs = OFFS[:].rearrange("p (c e) -> p c e", e=3)
                    nc.vector.tensor_tensor(dst, src[:, :, 0:K], ofs,
                                            op=AA.add)
                    # fold hop 1: contiguous dump [128, 2*CPS*K] -> DRAM
                    nc.sync.dma_start(idxscr[s, b], IDXC[:])
                    # fold hop 2: scramble on read into wrapped [lane][word]
                    # word (pg, cp, k); WIDX[16g+q, f] = stream i = f*16+q
                    srcw = idxscr[s, b].rearrange(
                        "(pg q) (th ck) -> q th pg ck",
                        pg=8, q=16, th=2, ck=CPS * K)
                    for th in range(2):
                        g = b * 2 + th
                        dst = WIDX[g * 16:(g + 1) * 16,
                                   WPS * s:WPS * (s + 1)].bitcast(U16).rearrange(
                            "q (pg ck) -> q pg ck", pg=8, ck=CPS * K)
                        nc.sync.dma_start(dst, srcw[:, th])
                    if s == 0:
                        emit_tables_b(b)

                # quarter-gather: all 4 batches, d=4 o-channels per index
                nc.gpsimd.ap_gather(
                    G[:, IPS * 4 * s:IPS * 4 * (s + 1)].rearrange(
                        "p (i j) -> p i j", j=4),
                    TABQ[:].rearrange("p (r j) -> p r j", j=4),
                    WIDX[:, WPS * s:WPS * (s + 1)],
                    channels=128, num_elems=K * M, d=4, num_idxs=IPS)

            # ---- k-sum per split: OUT[p,(cp,pg,pl,j)] = sum_k G[...] ----
            for s in range(NSPLIT):
                gv = G[:, IPS * 4 * s:IPS * 4 * (s + 1)].rearrange(
                    "p (pg cp k pl j) -> p (pg cp) k (pl j)",
                    pg=8, cp=CPS, k=K, pl=16, j=4)
                T1 = op_.tile([128, CPS * 8 * 64], BF16, tag="T1")
                t1v = T1[:].rearrange("p (t w) -> p t w", w=64)
                OUT = op_.tile([128, CPS * 8 * 64], BF16, tag="OUT")
                ov = OUT[:].rearrange("p (t w) -> p t w", w=64)
                nc.vector.tensor_tensor(t1v, gv[:, :, 0, :], gv[:, :, 1, :],
                                        op=AA.add)
                nc.vector.tensor_tensor(ov, t1v, gv[:, :, 2, :], op=AA.add)
                nc.sync.dma_start(
                    outd[:, CPS * 8 * 64 * s:CPS * 8 * 64 * (s + 1)], OUT[:])

    nc.compile()
    return nc


def host_prep(x, weight, bias):
    """Full inputs -> per-core in_maps (list of 8 dicts)."""
    x = np.ascontiguousarray(np.asarray(x), dtype=np.float32)
    weight = np.asarray(weight, dtype=np.float32)
    bias = np.asarray(bias, dtype=np.float32)
    B = x.shape[0]
    x1 = x.reshape(B, 16, 64, 2, 64, 2).transpose(0, 1, 3, 5, 2, 4)
    x2 = np.ascontiguousarray(x1).reshape(B, C1, N)
    xs = np.ascontiguousarray(x2[:, :, SAMPLE_FLAT])

    # hi/lo bf16 splits of X65 = [x2; 1] and S65 = [2*xs; -m2]
    X65 = np.empty((B, 65, N), np.float32)
    X65[:, :64] = x2
    X65[:, 64] = 1.0
    xh = X65.astype(ml_dtypes.bfloat16)
    xl = (X65 - xh.astype(np.float32)).astype(ml_dtypes.bfloat16)
    m2 = np.einsum("bcm,bcm->bm", xs, xs, dtype=np.float32)
    S65 = np.empty((B, 65, M), np.float32)
    S65[:, :64] = xs * np.float32(2.0)
    S65[:, 64] = -m2
    sh = S65.astype(ml_dtypes.bfloat16)
    sl = (S65 - sh.astype(np.float32)).astype(ml_dtypes.bfloat16)

    xse = np.zeros((B, 66, M), np.float32)
    xse[:, :64] = xs * np.float32(2.0)
    xse[:, 65] = 1.0
    # padded o''-permutation: col o'' = j*32 + oq holds W row oq*4+j
    opp = np.arange(128)
    jj, oq = opp // 32, opp % 32
    valid = oq < 16
    orow = np.where(valid, (oq % 16) * 4 + jj, 0)
    wke = np.zeros((K, 66, 128), np.float32)
    for k in range(K):
        wke[k, :64] = np.where(valid[None, :],
                               weight[orow, :, k].T * np.float32(0.5), 0.0)
        wke[k, 65] = np.where(valid, bias[orow] * np.float32(1.0 / 3.0), 0.0)
    # delta fold: dle[b][o''][p] = (o''%32 == p%16) for p in b-block
    dle = np.zeros((B_PER_CORE, 128, 128), ml_dtypes.bfloat16)
    pp = np.arange(128)
    eq = (opp[:, None] % 32) == (pp[None, :] % 16)
    for b in range(B_PER_CORE):
        blk = (pp >= b * 32) & (pp < (b + 1) * 32)
        dle[b] = (eq & blk[None, :]).astype(ml_dtypes.bfloat16)
    offe = np.tile(np.arange(K, dtype=np.uint16) * 256, 2 * CPS)
    offe = np.broadcast_to(offe, (128, 2 * CPS * K)).copy()

    in_maps = []
    for core in range(N_CORES):
        sl_ = slice(core * B_PER_CORE, (core + 1) * B_PER_CORE)
        in_maps.append({
            "xhe": np.ascontiguousarray(xh[sl_]),
            "xle": np.ascontiguousarray(xl[sl_]),
            "she": np.ascontiguousarray(sh[sl_]),
            "sle": np.ascontiguousarray(sl[sl_]),
            "xse": np.ascontiguousarray(xse[sl_]),
            "wke": wke,
            "dle": dle,
            "offe": offe,
        })
    return in_maps


def host_post(results):
    """Per-core outd [128, 8192] bf16 -> full output [32, 16, 128, 128] f32."""
    B = N_CORES * B_PER_CORE
    out = np.empty((B, C1, N), np.float32)
    for core in range(N_CORES):
        o = np.asarray(results[core]["outd"]).astype(np.float32)
        # partition = (b, th, oq); free = (s, pg, cp, pl, j); o = oq*4+j
        arr = o.reshape(B_PER_CORE, 2, 16, NSPLIT, 8, CPS, 16, 4)
        # out[b, oq*4+j, th*2048 + (s*CPS+cp)*128 + pg*16 + pl]
        blk = (arr.transpose(0, 2, 7, 1, 3, 5, 4, 6)  # b oq j th s cp pg pl
               .reshape(B_PER_CORE, C1, N))
        out[core * B_PER_CORE:(core + 1) * B_PER_CORE] = blk
    out = out.reshape(B, C1, 64, 64)
    y = (out.reshape(B, 16, 2, 2, 64, 64).transpose(0, 1, 4, 2, 5, 3)
         .reshape(B, 16, 128, 128))
    return np.ascontiguousarray(y)


def kernel(x, weight, bias):
    if "nc" not in _CACHE:
        _CACHE["nc"] = build_program()
    nc = _CACHE["nc"]
    in_maps = host_prep(x, weight, bias)
    res = run_bass_kernel_spmd(nc, in_maps, core_ids=list(range(N_CORES)))
    return host_post(res.results)
